# revision 28
# baseline (speedup 1.0000x reference)
"""Longformer sliding-window self-attention (B=2, S=4096, D=768, H=12, Dh=64,
one-sided window W=256) on 8 TRN2 NeuronCores.

Sharding: (batch, head-group) - core = b*4 + g handles batch b, heads
[3g, 3g+3). Each core runs the same SPMD Bass program on its shard.

v2 layout (all-bf16 datapath, f32 PSUM accumulation):

  phase 1: X (bf16, host-cast) -> X^T via PE transpose (1 cyc/row);
           stacked projection W_all [768, 576] bf16 (cols:
           q01|k01|k2+v2|v01|q2) so every matmul stream has a full
           128-row output tile (except the last 64); Q^T/K^T evacuated
           as bf16 packed two-heads-per-128-partitions for row-group
           packed score matmuls; V re-transposed into V_aug
           [s, 3*(64+1)] with a ones column (fused softmax denom).
  phase 2: per 256-query chunk: banded scores S^T[k, q] with head0/1
           packed into concurrent 64-row-group matmuls (contraction is
           only Dh=64); exp on ACT per 2-bank PSUM tile; band masking
           via triangular 0/1 multiplies split DVE (h01, strided) /
           GpSimd (h2); O^T = P^T.T @ V_aug accumulated per key tile
           (bf16 stationary -> FWL); epilogue 1/Z scaling on DVE.

kernel() takes full inputs, shards, runs SPMD on cores 0..7, reassembles.
"""
import sys

if '/opt/trn_rl_repo' not in sys.path:
    sys.path.insert(0, '/opt/trn_rl_repo')

import math
from contextlib import ExitStack

import numpy as np
import ml_dtypes

import concourse.bacc as bacc
import concourse.mybir as mybir
import concourse.tile as tile
from concourse.bass_utils import run_bass_kernel_spmd

F32 = mybir.dt.float32
F32R = mybir.dt.float32r
BF16 = mybir.dt.bfloat16

B, S, D = 2, 4096, 768
H, DH, W = 12, 64, 256
HPC = 3              # heads per core
DHC = HPC * DH       # 192 head-dims per core
NCORES = 8
C2 = 256             # query chunk
NCH = S // C2        # 16 chunks
NKT = S // 128       # 32 key tiles
SBLK = 512           # projection s-block
NSB = S // SBLK      # 8 s-blocks
VAW = DH + 1         # 65: V columns + ones column
WCOLS = 576          # stacked projection output dims
AluOp = mybir.AluOpType
ActFn = mybir.ActivationFunctionType

# stacked W col ranges (per 128-tile):
#   t0 [0:128]   = Wq h0|h1      -> qTp
#   t1 [128:256] = Wk h0|h1      -> kTp
#   t2 [256:384] = Wk h2 | Wv h2 -> k2t (rows 0:64), v2sb (rows 64:128)
#   t3 [384:512] = Wv h0|h1      -> v01sb
#   t4 [512:576] = Wq h2         -> q2t
MT_SLICES = [(0, 128), (128, 128), (256, 128), (384, 128), (512, 64)]

# feature toggles (HW-bisect knobs)
FANCY_MASKS = True    # strided+broadcast DVE mask ops
SPLIT_BANK = True     # h0/h1 packed matmul outputs in different PSUM banks


def _build_program(use_fmask, use_qmask, use_bias, debug=False):
    nc = bacc.Bacc("TRN2", num_devices=NCORES)

    x_d = nc.dram_tensor("x", (S, D), BF16, kind="ExternalInput").ap()
    wall_d = nc.dram_tensor("wall", (D, WCOLS), BF16, kind="ExternalInput").ap()
    idb_d = nc.dram_tensor("identb", (128, 128), BF16, kind="ExternalInput").ap()
    idblo_d = nc.dram_tensor("identblo", (128, 64), BF16, kind="ExternalInput").ap()
    tge_d = nc.dram_tensor("t_ge", (128, 128), BF16, kind="ExternalInput").ap()
    tle_d = nc.dram_tensor("t_le", (128, 128), BF16, kind="ExternalInput").ap()
    if use_bias:
        ball_d = nc.dram_tensor("ball", (WCOLS, 1), F32, kind="ExternalInput").ap()
    if use_fmask:
        fmk_d = nc.dram_tensor("fmk", (128, NKT), F32, kind="ExternalInput").ap()
    if use_qmask:
        qmk_d = nc.dram_tensor("qmk", (128, NKT), F32, kind="ExternalInput").ap()
    out_d = nc.dram_tensor("out", (S, DHC), F32, kind="ExternalOutput").ap()
    if debug:
        dbg_qTp = nc.dram_tensor("dbg_qTp", (128, S), BF16, kind="ExternalOutput").ap()
        dbg_kTp = nc.dram_tensor("dbg_kTp", (128, S), BF16, kind="ExternalOutput").ap()
        dbg_q2t = nc.dram_tensor("dbg_q2t", (64, S), BF16, kind="ExternalOutput").ap()
        dbg_k2t = nc.dram_tensor("dbg_k2t", (64, S), BF16, kind="ExternalOutput").ap()
        dbg_va = nc.dram_tensor("dbg_va", (128, NKT * HPC * VAW), BF16,
                                kind="ExternalOutput").ap()
        dbg_pt01 = nc.dram_tensor("dbg_pt01", (128, 6 * 2 * C2), BF16,
                                  kind="ExternalOutput").ap()
        dbg_pt2 = nc.dram_tensor("dbg_pt2", (128, 6 * C2), BF16,
                                 kind="ExternalOutput").ap()
        dbg_av = nc.dram_tensor("dbg_av", (128, 6 * VAW), F32,
                                kind="ExternalOutput").ap()
        DBG_CI = 5

    with tile.TileContext(nc) as tc, ExitStack() as ctx:
        pers = ctx.enter_context(tc.tile_pool(name="pers", bufs=1))

        # prefetch order matters: the first transposes need identb + x(sb0),
        # so issue those DMAs before the bulkier weight/constant loads.
        identb = pers.tile([128, 128], BF16, tag="identb", name="identb")
        nc.sync.dma_start(identb[:], idb_d)
        xin0 = pers.tile([128, 4 * D], BF16, tag="xin0", name="xin0")
        xin0_3 = xin0.rearrange("p (a d) -> p a d", a=4)
        nc.sync.dma_start(
            xin0_3[:], x_d[0:SBLK, :].rearrange("(a p) d -> p a d", p=128))
        w_sb = pers.tile([128, 6 * WCOLS], BF16, tag="w", name="w")
        nc.sync.dma_start(w_sb[:], wall_d.rearrange("(a p) n -> p a n", p=128))
        identblo = pers.tile([128, 64], BF16, tag="identblo", name="identblo")
        nc.sync.dma_start(identblo[:], idblo_d)
        t_ge = pers.tile([128, 128], BF16, tag="t_ge", name="t_ge")
        t_le = pers.tile([128, 128], BF16, tag="t_le", name="t_le")
        nc.sync.dma_start(t_ge[:], tge_d)
        nc.sync.dma_start(t_le[:], tle_d)
        if use_bias:
            ball = pers.tile([128, 5], F32, tag="ball", name="ball")
            # per mt-tile bias column: ball[:, mt] rows 0:msz
            nc.sync.dma_start(
                ball[:, 0:4],
                ball_d[0:512, :].rearrange("(a p) o -> p (a o)", p=128))
            nc.sync.dma_start(ball[0:64, 4:5], ball_d[512:WCOLS, :])
        if use_fmask:
            fmk = pers.tile([128, NKT], F32, tag="fmk", name="fmk")
            nc.sync.dma_start(fmk[:], fmk_d)
        if use_qmask:
            qmk = pers.tile([128, NKT], F32, tag="qmk", name="qmk")
            nc.sync.dma_start(qmk[:], qmk_d)

        # persistent activations
        qTp = pers.tile([128, S], BF16, tag="qTp", name="qTp")
        kTp = pers.tile([128, S], BF16, tag="kTp", name="kTp")
        q2t = pers.tile([64, S], BF16, tag="q2t", name="q2t")
        k2t = pers.tile([64, S], BF16, tag="k2t", name="k2t")
        va = pers.tile([128, NKT * HPC * VAW], BF16, tag="va", name="va")
        va4 = va.rearrange("p (t h c) -> p t h c", h=HPC, c=VAW)
        nc.gpsimd.memset(va4[:, :, :, DH:VAW], 1.0)

        # ---------------- phase 1: X^T, projections, V_aug ----------------
        with tc.tile_pool(name="p1s", bufs=2) as p1s, \
             tc.tile_pool(name="pp_tp", bufs=3, space="PSUM") as pp_tp, \
             tc.tile_pool(name="pp_pj", bufs=3, space="PSUM") as pp_pj, \
             tc.tile_pool(name="pp_tv", bufs=2, space="PSUM") as pp_tv:
            for sb in range(NSB):
                sbc = slice(sb * SBLK, (sb + 1) * SBLK)
                if sb == 0:
                    xin3 = xin0_3
                else:
                    xin = p1s.tile([128, 4 * D], BF16, tag="xin", name="xin",
                                   bufs=2)
                    xin3 = xin.rearrange("p (a d) -> p a d", a=4)
                    nc.sync.dma_start(
                        xin3[:],
                        x_d[sbc, :].rearrange("(a p) d -> p a d", p=128))
                xt = [p1s.tile([128, SBLK], BF16, tag=f"xt{dt}", name=f"xt{dt}",
                               bufs=2) for dt in range(6)]
                for dt in range(6):
                    tp = pp_tp.tile([128, SBLK], BF16, tag="tp", name="tp",
                                    padded_shape=[128, 1024])
                    for st in range(4):
                        nc.tensor.transpose(
                            tp[:, st * 128:(st + 1) * 128],
                            xin3[:, st, dt * 128:(dt + 1) * 128], identb[:])
                    nc.scalar.activation(xt[dt][:], tp[:], ActFn.Copy)
                v01sb = p1s.tile([128, SBLK], BF16, tag="v01", name="v01", bufs=2)
                v2sb = p1s.tile([128, SBLK], BF16, tag="v2", name="v2", bufs=2)
                for mt, (m0, msz) in enumerate(MT_SLICES):
                    pj = pp_pj.tile([128, SBLK], F32, tag="pj", name="pj")
                    for kt in range(6):
                        nc.tensor.matmul(
                            pj[0:msz, :],
                            w_sb[:, kt * WCOLS + m0: kt * WCOLS + m0 + msz],
                            xt[kt][:],
                            start=(kt == 0), stop=(kt == 5))
                    if mt == 2:
                        dsts = [(pj[0:64, :], k2t[:, sbc], 0),
                                (pj[64:128, :], v2sb[64:128, :], 64)]
                    elif mt == 0:
                        dsts = [(pj[:, :], qTp[:, sbc], 0)]
                    elif mt == 1:
                        dsts = [(pj[:, :], kTp[:, sbc], 0)]
                    elif mt == 3:
                        dsts = [(pj[:, :], v01sb[:, :], 0)]
                    else:
                        dsts = [(pj[0:64, :], q2t[:, sbc], 0)]
                    for src, dst, r0 in dsts:
                        if use_bias:
                            nc.vector.tensor_scalar_add(
                                dst, src,
                                ball[r0:r0 + src.partition_size(), mt:mt + 1])
                        else:
                            nc.vector.tensor_copy(dst, src)
                # V_aug for this block's 4 s-tiles, grouped per head
                for h in range(HPC):
                    tv = pp_tv.tile([128, 4 * DH], BF16, tag="tv", name="tv",
                                    padded_shape=[128, 1024])
                    for st in range(4):
                        if h == 0:
                            src = v01sb[0:64, st * 128:(st + 1) * 128]
                            idn = identb[0:64, 0:64]
                        elif h == 1:
                            src = v01sb[64:128, st * 128:(st + 1) * 128]
                            idn = identblo[64:128, :]
                        else:
                            src = v2sb[64:128, st * 128:(st + 1) * 128]
                            idn = identblo[64:128, :]
                        nc.tensor.transpose(
                            tv[:, st * DH:(st + 1) * DH], src, idn)
                    nc.vector.tensor_copy(
                        va4[:, sb * 4:(sb + 1) * 4, h, 0:DH],
                        tv.rearrange("p (a d) -> p a d", a=4))

        if debug:
            nc.sync.dma_start(dbg_qTp, qTp[:])
            nc.sync.dma_start(dbg_kTp, kTp[:])
            nc.sync.dma_start(dbg_q2t, q2t[:])
            nc.sync.dma_start(dbg_k2t, k2t[:])
            nc.sync.dma_start(dbg_va, va[:])

        # ---------------- phase 2: banded attention ----------------
        with tc.tile_pool(name="p2s", bufs=1) as p2s, \
             tc.tile_pool(name="pp_sc", bufs=2, space="PSUM") as pp_sc, \
             tc.tile_pool(name="pp_ot", bufs=1, space="PSUM") as pp_ot, \
             tc.tile_pool(name="pp_av", bufs=2, space="PSUM") as pp_av:
            for ci in range(NCH):
                cic = slice(ci * C2, (ci + 1) * C2)
                kt0 = max(0, 2 * ci - 2)
                kt1 = min(NKT - 1, 2 * ci + 3)
                nkt = kt1 - kt0 + 1
                pt01 = p2s.tile([128, 6 * 2 * C2], BF16, tag="pt01",
                                name="pt01", bufs=3)
                pt2 = p2s.tile([128, 6 * C2], BF16, tag="pt2", name="pt2",
                               bufs=3)
                pt01_4 = pt01.rearrange("p (t g c) -> p t g c", g=4, c=128)
                pt2_2 = pt2.rearrange("p (t g c) -> p t g c", g=2, c=128)

                # h0/h1 scores, packed in 64-row groups; exp per 2-kt tile
                for i0 in range(0, nkt, 2):
                    n = min(2, nkt - i0)
                    sc = pp_sc.tile([128, 1024], F32, tag="sc", name="sc")
                    for kk in range(n):
                        kt = kt0 + i0 + kk
                        ktc = slice(kt * 128, (kt + 1) * 128)
                        if SPLIT_BANK:
                            o0 = sc[:, kk * 256:(kk + 1) * 256]
                            o1 = sc[:, 512 + kk * 256: 512 + (kk + 1) * 256]
                        else:
                            o0 = sc[:, kk * 512: kk * 512 + 256]
                            o1 = sc[:, kk * 512 + 256:(kk + 1) * 512]
                        nc.tensor.matmul(o0, kTp[0:64, ktc], qTp[0:64, cic],
                                         start=True, stop=True)
                        nc.tensor.matmul(o1, kTp[64:128, ktc],
                                         qTp[64:128, cic],
                                         start=True, stop=True)
                    if SPLIT_BANK:
                        sc4 = sc.rearrange("p (h k c) -> p h k c",
                                           h=2, k=2, c=256)
                        dst = pt01[:, i0 * 512:(i0 + n) * 512]
                        if n == 2:
                            nc.scalar.activation(
                                dst.rearrange("p (k h c) -> p k h c",
                                              k=2, h=2, c=256),
                                sc4.rearrange("p h k c -> p k h c"),
                                ActFn.Exp)
                        else:
                            nc.scalar.activation(
                                dst.rearrange("p (h c) -> p h c", h=2, c=256),
                                sc4[:, :, 0, :], ActFn.Exp)
                    else:
                        nc.scalar.activation(
                            pt01[:, i0 * 512:(i0 + n) * 512],
                            sc[:, 0:n * 512], ActFn.Exp)
                # h2 scores: up to 4 kt per 2-bank tile
                for i0 in range(0, nkt, 4):
                    n = min(4, nkt - i0)
                    sc = pp_sc.tile([128, 1024], F32, tag="sc", name="sc")
                    for kk in range(n):
                        kt = kt0 + i0 + kk
                        ktc = slice(kt * 128, (kt + 1) * 128)
                        nc.tensor.matmul(
                            sc[:, kk * 256:(kk + 1) * 256],
                            k2t[0:64, ktc], q2t[0:64, cic],
                            start=True, stop=True)
                    nc.scalar.activation(
                        pt2[:, i0 * 256:(i0 + n) * 256], sc[:, 0:n * 256],
                        ActFn.Exp)

                # band masks: DVE for h01, GpSimd for h2
                if FANCY_MASKS:
                    tge_b = t_ge[:].rearrange("p (x c) -> p x c", x=1) \
                        .broadcast_to([128, 2, 128])
                    tle_b = t_le[:].rearrange("p (x c) -> p x c", x=1) \
                        .broadcast_to([128, 2, 128])

                def _mask01(i, hf, mt):
                    if FANCY_MASKS:
                        sl = pt01_4[:, i, hf:hf + 3:2, :]
                        nc.vector.tensor_tensor(sl, sl,
                                                tge_b if mt == 0 else tle_b,
                                                op=AluOp.mult)
                    else:
                        m = t_ge[:] if mt == 0 else t_le[:]
                        for g in (hf, hf + 2):
                            sl = pt01_4[:, i, g, :]
                            nc.vector.tensor_tensor(sl, sl, m, op=AluOp.mult)

                for kt in range(kt0, kt1 + 1):
                    j = kt - 2 * ci
                    i = kt - kt0
                    if j == -2:
                        _mask01(i, 0, 0)
                        s2 = pt2_2[:, i, 0, :]
                        nc.gpsimd.tensor_tensor(s2, s2, t_ge[:], op=AluOp.mult)
                    elif j == -1:
                        _mask01(i, 1, 0)
                        s2 = pt2_2[:, i, 1, :]
                        nc.gpsimd.tensor_tensor(s2, s2, t_ge[:], op=AluOp.mult)
                    elif j == 2:
                        _mask01(i, 0, 1)
                        s2 = pt2_2[:, i, 0, :]
                        nc.gpsimd.tensor_tensor(s2, s2, t_le[:], op=AluOp.mult)
                    elif j == 3:
                        _mask01(i, 1, 1)
                        s2 = pt2_2[:, i, 1, :]
                        nc.gpsimd.tensor_tensor(s2, s2, t_le[:], op=AluOp.mult)
                    if use_fmask:
                        nc.vector.tensor_scalar_mul(
                            pt01[:, i * 512:(i + 1) * 512],
                            pt01[:, i * 512:(i + 1) * 512], fmk[:, kt:kt + 1])
                        nc.gpsimd.tensor_scalar_mul(
                            pt2[:, i * 256:(i + 1) * 256],
                            pt2[:, i * 256:(i + 1) * 256], fmk[:, kt:kt + 1])

                if debug and ci == DBG_CI:
                    nc.sync.dma_start(dbg_pt01, pt01[:])
                    nc.sync.dma_start(dbg_pt2, pt2[:])

                # zero the fully-out-of-band query halves (exp skipped them
                # but the wide V-stationary AV matmul reads all 256 q cols)
                for kt in (kt0, kt1):
                    j = kt - 2 * ci
                    i = kt - kt0
                    if j == -2:
                        nc.vector.memset(pt01_4[:, i, 1:4:2, :], 0.0)
                        nc.gpsimd.memset(pt2_2[:, i, 1, :], 0.0)
                    elif j == 3:
                        nc.vector.memset(pt01_4[:, i, 0:3:2, :], 0.0)
                        nc.gpsimd.memset(pt2_2[:, i, 0, :], 0.0)

                # AV (V-stationary, dense): O^T[65, q] = sum_kt va.T @ P_kt,
                # then PE-transpose the two 128-query halves back to [q, 65].
                # h0+h1 share one 1-bank oT tile (one DVE evac for both).
                avT = pp_av.tile([128, 6 * 68], BF16, tag="avT", name="avT",
                                 padded_shape=[128, 1024])
                avT3 = avT.rearrange("p (g c) -> p g c", c=68)
                oT01 = pp_ot.tile([65, 2 * C2], F32, tag="oT01", name="oT01",
                                  padded_shape=[128, 512])
                oT2 = pp_ot.tile([65, C2], F32, tag="oT2", name="oT2",
                                 padded_shape=[128, 512])
                for h in range(HPC):
                    oT = oT2[:, :] if h == 2 else oT01[:, h * C2:(h + 1) * C2]
                    for i in range(nkt):
                        kt = kt0 + i
                        if h < 2:
                            rhs = pt01[:, i * 512 + 256 * h:
                                       i * 512 + 256 * h + 256]
                        else:
                            rhs = pt2[:, i * 256:(i + 1) * 256]
                        nc.tensor.matmul(
                            oT, va4[:, kt, h, :], rhs,
                            start=(i == 0), stop=(i == nkt - 1))
                oT_sb = p2s.tile([65, 2 * C2], BF16, tag="otsb",
                                 name="oT_sb", bufs=2)
                oT2_sb = p2s.tile([65, C2], BF16, tag="ot2sb",
                                  name="oT2_sb", bufs=2)
                nc.vector.tensor_copy(oT_sb[:], oT01[:])
                nc.vector.tensor_copy(oT2_sb[:], oT2[:])
                for h in range(HPC):
                    for hf in range(2):
                        g = h * 2 + hf
                        src = (oT2_sb[:, hf * 128:(hf + 1) * 128] if h == 2
                               else oT_sb[:, h * C2 + hf * 128:
                                          h * C2 + (hf + 1) * 128])
                        nc.tensor.transpose(
                            avT3[:, g, 0:VAW], src, identb[0:65, 0:65])

                if debug and ci == DBG_CI:
                    av_sb = p2s.tile([128, 6 * VAW], F32, tag="avsb",
                                     name="av_sb", bufs=1)
                    nc.vector.tensor_copy(
                        av_sb.rearrange("p (g c) -> p g c", c=VAW),
                        avT3[:, :, 0:VAW])
                    nc.sync.dma_start(dbg_av, av_sb[:])

                # epilogue on DVE: reciprocal of Z, scale, store
                rzs = p2s.tile([128, 6], F32, tag="rzs", name="rzs", bufs=3)
                nc.vector.reciprocal(rzs[:], avT3[:, :, DH])
                if use_qmask:
                    for g in range(6):
                        nc.vector.tensor_scalar_mul(
                            rzs[:, g:g + 1], rzs[:, g:g + 1],
                            qmk[:, 2 * ci + (g % 2):2 * ci + (g % 2) + 1])
                os_t = [p2s.tile([128, DHC], F32, tag=f"os{hf}",
                                 name=f"os{hf}", bufs=3) for hf in range(2)]
                for h in range(HPC):
                    for hf in range(2):
                        g = h * 2 + hf
                        nc.vector.tensor_scalar_mul(
                            os_t[hf][:, h * DH:(h + 1) * DH],
                            avT3[:, g, 0:DH], rzs[:, g:g + 1])
                for hf in range(2):
                    qt = 2 * ci + hf
                    nc.sync.dma_start(
                        out_d[qt * 128:(qt + 1) * 128, :], os_t[hf][:])

    nc.compile()
    return nc


_prog_cache = {}


def _get_program(use_fmask, use_qmask, use_bias):
    key = (use_fmask, use_qmask, use_bias)
    if key not in _prog_cache:
        _prog_cache[key] = _build_program(use_fmask, use_qmask, use_bias)
    return _prog_cache[key]


def _host_constants():
    kl = np.arange(128)[:, None]
    ql = np.arange(128)[None, :]
    bf = ml_dtypes.bfloat16
    t_ge = (kl >= ql).astype(bf)
    t_le = (kl <= ql).astype(bf)
    identb = np.eye(128, dtype=np.float32).astype(bf)
    identblo = np.zeros((128, 64), dtype=np.float32)
    identblo[64:128, :] = np.eye(64, dtype=np.float32)
    identblo = identblo.astype(bf)
    return identb, identblo, t_ge, t_le


def kernel(hidden_states, attention_mask, is_index_masked, Wq, bq, Wk, bk, Wv, bv,
           trace=False):
    hidden_states = np.asarray(hidden_states, dtype=np.float32)
    attention_mask = np.asarray(attention_mask, dtype=np.float32)
    is_index_masked = np.asarray(is_index_masked)
    Wq = np.asarray(Wq, dtype=np.float32)
    Wk = np.asarray(Wk, dtype=np.float32)
    Wv = np.asarray(Wv, dtype=np.float32)
    bq = np.asarray(bq, dtype=np.float32)
    bk = np.asarray(bk, dtype=np.float32)
    bv = np.asarray(bv, dtype=np.float32)

    use_fmask = bool(np.any(attention_mask != 0))
    use_qmask = bool(np.any(is_index_masked))
    use_bias = bool(np.any(bq != 0) or np.any(bk != 0) or np.any(bv != 0))
    nc = _get_program(use_fmask, use_qmask, use_bias)

    scale = 1.0 / math.sqrt(DH)
    identb, identblo, t_ge, t_le = _host_constants()
    bf = ml_dtypes.bfloat16
    x_bf = [np.ascontiguousarray(hidden_states[b]).astype(bf) for b in range(B)]

    in_maps = []
    for cid in range(NCORES):
        b = cid // 4
        h0 = HPC * (cid % 4)
        c = [slice((h0 + hh) * DH, (h0 + hh + 1) * DH) for hh in range(HPC)]
        wall = np.empty((D, WCOLS), dtype=np.float32)
        wall[:, 0:64] = Wq[:, c[0]] * scale
        wall[:, 64:128] = Wq[:, c[1]] * scale
        wall[:, 128:192] = Wk[:, c[0]]
        wall[:, 192:256] = Wk[:, c[1]]
        wall[:, 256:320] = Wk[:, c[2]]
        wall[:, 320:384] = Wv[:, c[2]]
        wall[:, 384:448] = Wv[:, c[0]]
        wall[:, 448:512] = Wv[:, c[1]]
        wall[:, 512:576] = Wq[:, c[2]] * scale
        m = {
            "x": x_bf[b],
            "wall": np.ascontiguousarray(wall.astype(bf)),
            "identb": identb,
            "identblo": identblo,
            "t_ge": t_ge,
            "t_le": t_le,
        }
        if use_bias:
            ball = np.empty((WCOLS, 1), dtype=np.float32)
            ball[0:64, 0] = bq[c[0]] * scale
            ball[64:128, 0] = bq[c[1]] * scale
            ball[128:192, 0] = bk[c[0]]
            ball[192:256, 0] = bk[c[1]]
            ball[256:320, 0] = bk[c[2]]
            ball[320:384, 0] = bv[c[2]]
            ball[384:448, 0] = bv[c[0]]
            ball[448:512, 0] = bv[c[1]]
            ball[512:576, 0] = bq[c[2]] * scale
            m["ball"] = ball
        if use_fmask:
            fac = (attention_mask[b] == 0).astype(np.float32)
            m["fmk"] = np.ascontiguousarray(fac.reshape(NKT, 128).T)
        if use_qmask:
            keep = (~is_index_masked[b]).astype(np.float32)
            m["qmk"] = np.ascontiguousarray(keep.reshape(NKT, 128).T)
        in_maps.append(m)

    res = run_bass_kernel_spmd(nc, in_maps, core_ids=list(range(NCORES)),
                               trace=trace)
    out = np.empty((B, S, D), dtype=np.float32)
    for cid in range(NCORES):
        b = cid // 4
        h0 = HPC * (cid % 4)
        out[b, :, h0 * DH:(h0 + HPC) * DH] = res.results[cid]["out"]
    if trace:
        return out, res
    return out


# revision 29
# speedup vs baseline: 1.2345x; 1.2345x over previous
"""Longformer sliding-window self-attention (B=2, S=4096, D=768, H=12, Dh=64,
one-sided window W=256) on 8 TRN2 NeuronCores.

Sharding: (batch, head-group) - core = b*4 + g handles batch b, heads
[3g, 3g+3). Each core runs the same SPMD Bass program on its shard.

v2 layout (all-bf16 datapath, f32 PSUM accumulation):

  phase 1: X (bf16, host-cast) -> X^T via PE transpose (1 cyc/row);
           stacked projection W_all [768, 576] bf16 (cols:
           q01|k01|k2+v2|v01|q2) so every matmul stream has a full
           128-row output tile (except the last 64); Q^T/K^T evacuated
           as bf16 packed two-heads-per-128-partitions for row-group
           packed score matmuls; V re-transposed into V_aug
           [s, 3*(64+1)] with a ones column (fused softmax denom).
  phase 2: per 256-query chunk: banded scores S^T[k, q] with head0/1
           packed into concurrent 64-row-group matmuls (contraction is
           only Dh=64); exp on ACT per 2-bank PSUM tile; band masking
           via triangular 0/1 multiplies split DVE (h01, strided) /
           GpSimd (h2); O^T = P^T.T @ V_aug accumulated per key tile
           (bf16 stationary -> FWL); epilogue 1/Z scaling on DVE.

kernel() takes full inputs, shards, runs SPMD on cores 0..7, reassembles.
"""
import sys

if '/opt/trn_rl_repo' not in sys.path:
    sys.path.insert(0, '/opt/trn_rl_repo')

import math
from contextlib import ExitStack

import numpy as np
import ml_dtypes

import concourse.bacc as bacc
import concourse.mybir as mybir
import concourse.tile as tile
from concourse.bass_utils import run_bass_kernel_spmd

F32 = mybir.dt.float32
F32R = mybir.dt.float32r
BF16 = mybir.dt.bfloat16

B, S, D = 2, 4096, 768
H, DH, W = 12, 64, 256
HPC = 3              # heads per core
DHC = HPC * DH       # 192 head-dims per core
NCORES = 8
C2 = 256             # query chunk
NCH = S // C2        # 16 chunks
NKT = S // 128       # 32 key tiles
SBLK = 512           # projection s-block
NSB = S // SBLK      # 8 s-blocks
VAW = DH + 1         # 65: V columns + ones column
WCOLS = 576          # stacked projection output dims
AluOp = mybir.AluOpType
ActFn = mybir.ActivationFunctionType

# stacked W col ranges (per 128-tile):
#   t0 [0:128]   = Wq h0|h1      -> qTp
#   t1 [128:256] = Wk h0|h1      -> kTp
#   t2 [256:384] = Wk h2 | Wv h2 -> k2t (rows 0:64), v2sb (rows 64:128)
#   t3 [384:512] = Wv h0|h1      -> v01sb
#   t4 [512:576] = Wq h2         -> q2t
MT_SLICES = [(0, 128), (128, 128), (256, 128), (384, 128), (512, 64)]

# feature toggles (HW-bisect knobs)
FANCY_MASKS = True    # strided+broadcast DVE mask ops
SPLIT_BANK = True     # h0/h1 packed matmul outputs in different PSUM banks


def _build_program(use_fmask, use_qmask, use_bias, debug=False):
    nc = bacc.Bacc("TRN2", num_devices=NCORES)

    x_d = nc.dram_tensor("x", (S, D), BF16, kind="ExternalInput").ap()
    wall_d = nc.dram_tensor("wall", (D, WCOLS), BF16, kind="ExternalInput").ap()
    idb_d = nc.dram_tensor("identb", (128, 128), BF16, kind="ExternalInput").ap()
    idblo_d = nc.dram_tensor("identblo", (128, 64), BF16, kind="ExternalInput").ap()
    tge_d = nc.dram_tensor("t_ge", (128, 128), BF16, kind="ExternalInput").ap()
    tle_d = nc.dram_tensor("t_le", (128, 128), BF16, kind="ExternalInput").ap()
    if use_bias:
        ball_d = nc.dram_tensor("ball", (WCOLS, 1), F32, kind="ExternalInput").ap()
    if use_fmask:
        fmk_d = nc.dram_tensor("fmk", (128, NKT), F32, kind="ExternalInput").ap()
    if use_qmask:
        qmk_d = nc.dram_tensor("qmk", (128, NKT), F32, kind="ExternalInput").ap()
    out_d = nc.dram_tensor("out", (S, DHC), F32, kind="ExternalOutput").ap()
    if debug:
        dbg_qTp = nc.dram_tensor("dbg_qTp", (128, S), BF16, kind="ExternalOutput").ap()
        dbg_kTp = nc.dram_tensor("dbg_kTp", (128, S), BF16, kind="ExternalOutput").ap()
        dbg_q2t = nc.dram_tensor("dbg_q2t", (64, S), BF16, kind="ExternalOutput").ap()
        dbg_k2t = nc.dram_tensor("dbg_k2t", (64, S), BF16, kind="ExternalOutput").ap()
        dbg_va = nc.dram_tensor("dbg_va", (128, NKT * HPC * VAW), BF16,
                                kind="ExternalOutput").ap()
        dbg_pt01 = nc.dram_tensor("dbg_pt01", (128, 6 * 2 * C2), BF16,
                                  kind="ExternalOutput").ap()
        dbg_pt2 = nc.dram_tensor("dbg_pt2", (128, 6 * C2), BF16,
                                 kind="ExternalOutput").ap()
        dbg_av = nc.dram_tensor("dbg_av", (128, 6 * VAW), F32,
                                kind="ExternalOutput").ap()
        DBG_CI = 5

    with tile.TileContext(nc) as tc, ExitStack() as ctx:
        pers = ctx.enter_context(tc.tile_pool(name="pers", bufs=1))

        # prefetch order matters: the first transposes need identb + x(sb0),
        # so issue those DMAs before the bulkier weight/constant loads.
        identb = pers.tile([128, 128], BF16, tag="identb", name="identb")
        nc.sync.dma_start(identb[:], idb_d)
        xin0 = pers.tile([128, 4 * D], BF16, tag="xin0", name="xin0")
        xin0_3 = xin0.rearrange("p (a d) -> p a d", a=4)
        nc.sync.dma_start(
            xin0_3[:], x_d[0:SBLK, :].rearrange("(a p) d -> p a d", p=128))
        w_sb = pers.tile([128, 6 * WCOLS], BF16, tag="w", name="w")
        nc.sync.dma_start(w_sb[:], wall_d.rearrange("(a p) n -> p a n", p=128))
        identblo = pers.tile([128, 64], BF16, tag="identblo", name="identblo")
        nc.sync.dma_start(identblo[:], idblo_d)
        t_ge = pers.tile([128, 128], BF16, tag="t_ge", name="t_ge")
        t_le = pers.tile([128, 128], BF16, tag="t_le", name="t_le")
        nc.sync.dma_start(t_ge[:], tge_d)
        nc.sync.dma_start(t_le[:], tle_d)
        if use_bias:
            ball = pers.tile([128, 5], F32, tag="ball", name="ball")
            # per mt-tile bias column: ball[:, mt] rows 0:msz
            nc.sync.dma_start(
                ball[:, 0:4],
                ball_d[0:512, :].rearrange("(a p) o -> p (a o)", p=128))
            nc.sync.dma_start(ball[0:64, 4:5], ball_d[512:WCOLS, :])
        if use_fmask:
            fmk = pers.tile([128, NKT], F32, tag="fmk", name="fmk")
            nc.sync.dma_start(fmk[:], fmk_d)
        if use_qmask:
            qmk = pers.tile([128, NKT], F32, tag="qmk", name="qmk")
            nc.sync.dma_start(qmk[:], qmk_d)

        # persistent activations
        qTp = pers.tile([128, S], BF16, tag="qTp", name="qTp")
        kTp = pers.tile([128, S], BF16, tag="kTp", name="kTp")
        q2t = pers.tile([64, S], BF16, tag="q2t", name="q2t")
        k2t = pers.tile([64, S], BF16, tag="k2t", name="k2t")
        va = pers.tile([128, NKT * HPC * VAW], BF16, tag="va", name="va")
        va4 = va.rearrange("p (t h c) -> p t h c", h=HPC, c=VAW)
        nc.gpsimd.memset(va4[:, :, :, DH:VAW], 1.0)

        # ---------------- phase 1: X^T, projections, V_aug ----------------
        with tc.tile_pool(name="p1s", bufs=2) as p1s, \
             tc.tile_pool(name="pp_tp", bufs=3, space="PSUM") as pp_tp, \
             tc.tile_pool(name="pp_pj", bufs=3, space="PSUM") as pp_pj, \
             tc.tile_pool(name="pp_tv", bufs=2, space="PSUM") as pp_tv:
            for sb in range(NSB):
                sbc = slice(sb * SBLK, (sb + 1) * SBLK)
                if sb == 0:
                    xin3 = xin0_3
                else:
                    xin = p1s.tile([128, 4 * D], BF16, tag="xin", name="xin",
                                   bufs=2)
                    xin3 = xin.rearrange("p (a d) -> p a d", a=4)
                    nc.sync.dma_start(
                        xin3[:],
                        x_d[sbc, :].rearrange("(a p) d -> p a d", p=128))
                xt = [p1s.tile([128, SBLK], BF16, tag=f"xt{dt}", name=f"xt{dt}",
                               bufs=2) for dt in range(6)]
                for dt in range(6):
                    tp = pp_tp.tile([128, SBLK], BF16, tag="tp", name="tp",
                                    padded_shape=[128, 1024])
                    for st in range(4):
                        nc.tensor.transpose(
                            tp[:, st * 128:(st + 1) * 128],
                            xin3[:, st, dt * 128:(dt + 1) * 128], identb[:])
                    nc.scalar.activation(xt[dt][:], tp[:], ActFn.Copy)
                v01sb = p1s.tile([128, SBLK], BF16, tag="v01", name="v01", bufs=2)
                v2sb = p1s.tile([128, SBLK], BF16, tag="v2", name="v2", bufs=2)
                for mt, (m0, msz) in enumerate(MT_SLICES):
                    pj = pp_pj.tile([128, SBLK], F32, tag="pj", name="pj")
                    for kt in range(6):
                        nc.tensor.matmul(
                            pj[0:msz, :],
                            w_sb[:, kt * WCOLS + m0: kt * WCOLS + m0 + msz],
                            xt[kt][:],
                            start=(kt == 0), stop=(kt == 5))
                    if mt == 2:
                        dsts = [(pj[0:64, :], k2t[:, sbc], 0),
                                (pj[64:128, :], v2sb[64:128, :], 64)]
                    elif mt == 0:
                        dsts = [(pj[:, :], qTp[:, sbc], 0)]
                    elif mt == 1:
                        dsts = [(pj[:, :], kTp[:, sbc], 0)]
                    elif mt == 3:
                        dsts = [(pj[:, :], v01sb[:, :], 0)]
                    else:
                        dsts = [(pj[0:64, :], q2t[:, sbc], 0)]
                    for src, dst, r0 in dsts:
                        if use_bias:
                            nc.vector.tensor_scalar_add(
                                dst, src,
                                ball[r0:r0 + src.partition_size(), mt:mt + 1])
                        else:
                            nc.vector.tensor_copy(dst, src)
                # V_aug for this block's 4 s-tiles, grouped per head
                for h in range(HPC):
                    tv = pp_tv.tile([128, 4 * DH], BF16, tag="tv", name="tv",
                                    padded_shape=[128, 1024])
                    for st in range(4):
                        if h == 0:
                            src = v01sb[0:64, st * 128:(st + 1) * 128]
                            idn = identb[0:64, 0:64]
                        elif h == 1:
                            src = v01sb[64:128, st * 128:(st + 1) * 128]
                            idn = identblo[64:128, :]
                        else:
                            src = v2sb[64:128, st * 128:(st + 1) * 128]
                            idn = identblo[64:128, :]
                        nc.tensor.transpose(
                            tv[:, st * DH:(st + 1) * DH], src, idn)
                    nc.vector.tensor_copy(
                        va4[:, sb * 4:(sb + 1) * 4, h, 0:DH],
                        tv.rearrange("p (a d) -> p a d", a=4))

        if debug:
            nc.sync.dma_start(dbg_qTp, qTp[:])
            nc.sync.dma_start(dbg_kTp, kTp[:])
            nc.sync.dma_start(dbg_q2t, q2t[:])
            nc.sync.dma_start(dbg_k2t, k2t[:])
            nc.sync.dma_start(dbg_va, va[:])

        # ---------------- phase 2: banded attention ----------------
        with tc.tile_pool(name="p2s", bufs=1) as p2s, \
             tc.tile_pool(name="pp_sc", bufs=3, space="PSUM") as pp_sc, \
             tc.tile_pool(name="pp_ot", bufs=1, space="PSUM") as pp_ot, \
             tc.tile_pool(name="pp_av", bufs=1, space="PSUM") as pp_av:
            for ci in range(NCH):
                cic = slice(ci * C2, (ci + 1) * C2)
                kt0 = max(0, 2 * ci - 2)
                kt1 = min(NKT - 1, 2 * ci + 3)
                nkt = kt1 - kt0 + 1
                pt01 = p2s.tile([128, 6 * 2 * C2], BF16, tag="pt01",
                                name="pt01", bufs=3)
                pt2 = p2s.tile([128, 6 * C2], BF16, tag="pt2", name="pt2",
                               bufs=3)
                pt01_4 = pt01.rearrange("p (t g c) -> p t g c", g=4, c=128)
                pt2_2 = pt2.rearrange("p (t g c) -> p t g c", g=2, c=128)

                # h0/h1 scores, packed in 64-row groups; exp per 2-kt tile
                for i0 in range(0, nkt, 2):
                    n = min(2, nkt - i0)
                    sc = pp_sc.tile([128, 1024], F32, tag="sc", name="sc")
                    for kk in range(n):
                        kt = kt0 + i0 + kk
                        ktc = slice(kt * 128, (kt + 1) * 128)
                        if SPLIT_BANK:
                            o0 = sc[:, kk * 256:(kk + 1) * 256]
                            o1 = sc[:, 512 + kk * 256: 512 + (kk + 1) * 256]
                        else:
                            o0 = sc[:, kk * 512: kk * 512 + 256]
                            o1 = sc[:, kk * 512 + 256:(kk + 1) * 512]
                        nc.tensor.matmul(o0, kTp[0:64, ktc], qTp[0:64, cic],
                                         start=True, stop=True)
                        nc.tensor.matmul(o1, kTp[64:128, ktc],
                                         qTp[64:128, cic],
                                         start=True, stop=True)
                    if SPLIT_BANK:
                        sc4 = sc.rearrange("p (h k c) -> p h k c",
                                           h=2, k=2, c=256)
                        dst = pt01[:, i0 * 512:(i0 + n) * 512]
                        if n == 2:
                            nc.scalar.activation(
                                dst.rearrange("p (k h c) -> p k h c",
                                              k=2, h=2, c=256),
                                sc4.rearrange("p h k c -> p k h c"),
                                ActFn.Exp)
                        else:
                            nc.scalar.activation(
                                dst.rearrange("p (h c) -> p h c", h=2, c=256),
                                sc4[:, :, 0, :], ActFn.Exp)
                    else:
                        nc.scalar.activation(
                            pt01[:, i0 * 512:(i0 + n) * 512],
                            sc[:, 0:n * 512], ActFn.Exp)
                # h2 scores: up to 4 kt per 2-bank tile
                for i0 in range(0, nkt, 4):
                    n = min(4, nkt - i0)
                    sc = pp_sc.tile([128, 1024], F32, tag="sc", name="sc")
                    for kk in range(n):
                        kt = kt0 + i0 + kk
                        ktc = slice(kt * 128, (kt + 1) * 128)
                        nc.tensor.matmul(
                            sc[:, kk * 256:(kk + 1) * 256],
                            k2t[0:64, ktc], q2t[0:64, cic],
                            start=True, stop=True)
                    nc.scalar.activation(
                        pt2[:, i0 * 256:(i0 + n) * 256], sc[:, 0:n * 256],
                        ActFn.Exp)

                # band masks: DVE for h01, GpSimd for h2
                if FANCY_MASKS:
                    tge_b = t_ge[:].rearrange("p (x c) -> p x c", x=1) \
                        .broadcast_to([128, 2, 128])
                    tle_b = t_le[:].rearrange("p (x c) -> p x c", x=1) \
                        .broadcast_to([128, 2, 128])

                def _mask01(i, hf, mt):
                    if FANCY_MASKS:
                        sl = pt01_4[:, i, hf:hf + 3:2, :]
                        nc.vector.tensor_tensor(sl, sl,
                                                tge_b if mt == 0 else tle_b,
                                                op=AluOp.mult)
                    else:
                        m = t_ge[:] if mt == 0 else t_le[:]
                        for g in (hf, hf + 2):
                            sl = pt01_4[:, i, g, :]
                            nc.vector.tensor_tensor(sl, sl, m, op=AluOp.mult)

                for kt in range(kt0, kt1 + 1):
                    j = kt - 2 * ci
                    i = kt - kt0
                    if j == -2:
                        _mask01(i, 0, 0)
                        s2 = pt2_2[:, i, 0, :]
                        nc.gpsimd.tensor_tensor(s2, s2, t_ge[:], op=AluOp.mult)
                    elif j == -1:
                        _mask01(i, 1, 0)
                        s2 = pt2_2[:, i, 1, :]
                        nc.gpsimd.tensor_tensor(s2, s2, t_ge[:], op=AluOp.mult)
                    elif j == 2:
                        _mask01(i, 0, 1)
                        s2 = pt2_2[:, i, 0, :]
                        nc.gpsimd.tensor_tensor(s2, s2, t_le[:], op=AluOp.mult)
                    elif j == 3:
                        _mask01(i, 1, 1)
                        s2 = pt2_2[:, i, 1, :]
                        nc.gpsimd.tensor_tensor(s2, s2, t_le[:], op=AluOp.mult)
                    if use_fmask:
                        nc.vector.tensor_scalar_mul(
                            pt01[:, i * 512:(i + 1) * 512],
                            pt01[:, i * 512:(i + 1) * 512], fmk[:, kt:kt + 1])
                        nc.gpsimd.tensor_scalar_mul(
                            pt2[:, i * 256:(i + 1) * 256],
                            pt2[:, i * 256:(i + 1) * 256], fmk[:, kt:kt + 1])

                if debug and ci == DBG_CI:
                    nc.sync.dma_start(dbg_pt01, pt01[:])
                    nc.sync.dma_start(dbg_pt2, pt2[:])

                # zero the fully-out-of-band query halves (exp skipped them
                # but the wide V-stationary AV matmul reads all 256 q cols)
                for kt in (kt0, kt1):
                    j = kt - 2 * ci
                    i = kt - kt0
                    if j == -2:
                        nc.vector.memset(pt01_4[:, i, 1:4:2, :], 0.0)
                        nc.gpsimd.memset(pt2_2[:, i, 1, :], 0.0)
                    elif j == 3:
                        nc.vector.memset(pt01_4[:, i, 0:3:2, :], 0.0)
                        nc.gpsimd.memset(pt2_2[:, i, 0, :], 0.0)

                # AV (V-stationary, dense): O^T[65, q] = sum_kt va.T @ P_kt,
                # then PE-transpose the two 128-query halves back to [q, 65]
                avT = pp_av.tile([128, 6 * 68], BF16, tag="avT", name="avT",
                                 padded_shape=[128, 1024])
                avT3 = avT.rearrange("p (g c) -> p g c", c=68)
                for h in range(HPC):
                    oT = pp_ot.tile([65, C2], F32, tag="oT", name="oT",
                                    padded_shape=[128, 512])
                    for i in range(nkt):
                        kt = kt0 + i
                        if h < 2:
                            rhs = pt01[:, i * 512 + 256 * h:
                                       i * 512 + 256 * h + 256]
                        else:
                            rhs = pt2[:, i * 256:(i + 1) * 256]
                        nc.tensor.matmul(
                            oT[:], va4[:, kt, h, :], rhs,
                            start=(i == 0), stop=(i == nkt - 1))
                    oT_sb = p2s.tile([65, C2], BF16, tag="otsb",
                                     name="oT_sb", bufs=2)
                    nc.vector.tensor_copy(oT_sb[:], oT[:])
                    for hf in range(2):
                        g = h * 2 + hf
                        nc.tensor.transpose(
                            avT3[:, g, 0:VAW],
                            oT_sb[:, hf * 128:(hf + 1) * 128],
                            identb[0:65, 0:65])

                if debug and ci == DBG_CI:
                    av_sb = p2s.tile([128, 6 * VAW], F32, tag="avsb",
                                     name="av_sb", bufs=1)
                    nc.vector.tensor_copy(
                        av_sb.rearrange("p (g c) -> p g c", c=VAW),
                        avT3[:, :, 0:VAW])
                    nc.sync.dma_start(dbg_av, av_sb[:])

                # epilogue on DVE: reciprocal of Z, scale, store
                rzs = p2s.tile([128, 6], F32, tag="rzs", name="rzs", bufs=3)
                nc.vector.reciprocal(rzs[:], avT3[:, :, DH])
                if use_qmask:
                    for g in range(6):
                        nc.vector.tensor_scalar_mul(
                            rzs[:, g:g + 1], rzs[:, g:g + 1],
                            qmk[:, 2 * ci + (g % 2):2 * ci + (g % 2) + 1])
                os_t = [p2s.tile([128, DHC], F32, tag=f"os{hf}",
                                 name=f"os{hf}", bufs=3) for hf in range(2)]
                for h in range(HPC):
                    for hf in range(2):
                        g = h * 2 + hf
                        nc.vector.tensor_scalar_mul(
                            os_t[hf][:, h * DH:(h + 1) * DH],
                            avT3[:, g, 0:DH], rzs[:, g:g + 1])
                for hf in range(2):
                    qt = 2 * ci + hf
                    nc.sync.dma_start(
                        out_d[qt * 128:(qt + 1) * 128, :], os_t[hf][:])

    nc.compile()
    return nc


_prog_cache = {}


def _get_program(use_fmask, use_qmask, use_bias):
    key = (use_fmask, use_qmask, use_bias)
    if key not in _prog_cache:
        _prog_cache[key] = _build_program(use_fmask, use_qmask, use_bias)
    return _prog_cache[key]


def _host_constants():
    kl = np.arange(128)[:, None]
    ql = np.arange(128)[None, :]
    bf = ml_dtypes.bfloat16
    t_ge = (kl >= ql).astype(bf)
    t_le = (kl <= ql).astype(bf)
    identb = np.eye(128, dtype=np.float32).astype(bf)
    identblo = np.zeros((128, 64), dtype=np.float32)
    identblo[64:128, :] = np.eye(64, dtype=np.float32)
    identblo = identblo.astype(bf)
    return identb, identblo, t_ge, t_le


def kernel(hidden_states, attention_mask, is_index_masked, Wq, bq, Wk, bk, Wv, bv,
           trace=False):
    hidden_states = np.asarray(hidden_states, dtype=np.float32)
    attention_mask = np.asarray(attention_mask, dtype=np.float32)
    is_index_masked = np.asarray(is_index_masked)
    Wq = np.asarray(Wq, dtype=np.float32)
    Wk = np.asarray(Wk, dtype=np.float32)
    Wv = np.asarray(Wv, dtype=np.float32)
    bq = np.asarray(bq, dtype=np.float32)
    bk = np.asarray(bk, dtype=np.float32)
    bv = np.asarray(bv, dtype=np.float32)

    use_fmask = bool(np.any(attention_mask != 0))
    use_qmask = bool(np.any(is_index_masked))
    use_bias = bool(np.any(bq != 0) or np.any(bk != 0) or np.any(bv != 0))
    nc = _get_program(use_fmask, use_qmask, use_bias)

    scale = 1.0 / math.sqrt(DH)
    identb, identblo, t_ge, t_le = _host_constants()
    bf = ml_dtypes.bfloat16
    x_bf = [np.ascontiguousarray(hidden_states[b]).astype(bf) for b in range(B)]

    in_maps = []
    for cid in range(NCORES):
        b = cid // 4
        h0 = HPC * (cid % 4)
        c = [slice((h0 + hh) * DH, (h0 + hh + 1) * DH) for hh in range(HPC)]
        wall = np.empty((D, WCOLS), dtype=np.float32)
        wall[:, 0:64] = Wq[:, c[0]] * scale
        wall[:, 64:128] = Wq[:, c[1]] * scale
        wall[:, 128:192] = Wk[:, c[0]]
        wall[:, 192:256] = Wk[:, c[1]]
        wall[:, 256:320] = Wk[:, c[2]]
        wall[:, 320:384] = Wv[:, c[2]]
        wall[:, 384:448] = Wv[:, c[0]]
        wall[:, 448:512] = Wv[:, c[1]]
        wall[:, 512:576] = Wq[:, c[2]] * scale
        m = {
            "x": x_bf[b],
            "wall": np.ascontiguousarray(wall.astype(bf)),
            "identb": identb,
            "identblo": identblo,
            "t_ge": t_ge,
            "t_le": t_le,
        }
        if use_bias:
            ball = np.empty((WCOLS, 1), dtype=np.float32)
            ball[0:64, 0] = bq[c[0]] * scale
            ball[64:128, 0] = bq[c[1]] * scale
            ball[128:192, 0] = bk[c[0]]
            ball[192:256, 0] = bk[c[1]]
            ball[256:320, 0] = bk[c[2]]
            ball[320:384, 0] = bv[c[2]]
            ball[384:448, 0] = bv[c[0]]
            ball[448:512, 0] = bv[c[1]]
            ball[512:576, 0] = bq[c[2]] * scale
            m["ball"] = ball
        if use_fmask:
            fac = (attention_mask[b] == 0).astype(np.float32)
            m["fmk"] = np.ascontiguousarray(fac.reshape(NKT, 128).T)
        if use_qmask:
            keep = (~is_index_masked[b]).astype(np.float32)
            m["qmk"] = np.ascontiguousarray(keep.reshape(NKT, 128).T)
        in_maps.append(m)

    res = run_bass_kernel_spmd(nc, in_maps, core_ids=list(range(NCORES)),
                               trace=trace)
    out = np.empty((B, S, D), dtype=np.float32)
    for cid in range(NCORES):
        b = cid // 4
        h0 = HPC * (cid % 4)
        out[b, :, h0 * DH:(h0 + HPC) * DH] = res.results[cid]["out"]
    if trace:
        return out, res
    return out
